# revision 1
# baseline (speedup 1.0000x reference)
"""Grouped per-sample MLP (conv1d groups=B) + GroupSwish + softmax, on 8 NeuronCores.

Data-parallel over the group/batch axis B=256: 32 groups per core.
Per group g: h = W1[g] @ x[g] + b1[g]; GroupSwish; o = W2[g] @ h + b2[g];
softmax over the flattened [C*L] logits.

Device strategy per core (per group, fully unrolled):
  - W1 matmul out[32, 512], contraction X=784 split 6x128 + 16, operands fed
    as float32r (TF32-like, 1 PE cycle/row, HW rounds internally) straight
    from DMA. fp32r matmuls must write PSUM at partition base 0.
  - GroupSwish via tanh (the only ACT table with both tanh and exp):
    (h+b1)*sigmoid(sp*(h+b1)) = ((h+b1)*0.5) * (1 + tanh(sp*(h+b1)/2)).
    The 1/1.1 factor is folded into W2 host-side; sp = softplus(beta) is
    computed on device via exp/ln.
  - Softmax without max-subtraction (logits are O(1)): exp with fused
    per-partition accum, cross-partition sum / broadcast via tiny matmuls
    against ones vectors.
"""

import os
import numpy as np
from contextlib import ExitStack

import concourse.mybir as mybir
import concourse.tile as tile
from concourse import bacc
from concourse.bass_utils import run_bass_kernel_spmd

B, X, Z, C, L = 256, 784, 32, 10, 512
NCORE = 8
GPC = B // NCORE  # 32 groups per core
NCH = 7  # K-chunks: 6*128 + 16
KLAST = X - 6 * 128  # 16
P = 128
F32 = mybir.dt.float32
F32R = mybir.dt.float32r

DEFAULT_CFG = dict(
    x_layout="interleave",  # "interleave": chunk c = rows 128c+p, 2KB runs;
    #                         "contig": one run/partition (uneven 7/6 rows)
    x_engines=("sync",),  # trigger engines for x loads, round-robin by group
    w_engine="sync",
    out_engine="gpsimd",
    const_engine="gpsimd",
    x_bufs=6,
    h_bufs=3,
    s_bufs=3,
    x_split=False,  # split each group's x-main DMA across sync+scalar queues
    x_pair=False,  # load two groups' x per DMA (halves trigger count)
    pipeline=False,  # defer W2 by one quad and softmax-normalize per quad,
    #                  two quads behind, to keep the PE stream stall-free
)

_CACHE: dict = {}


def _eng(nc, name):
    return getattr(nc, name)


def _build(cfg=DEFAULT_CFG):
    if cfg.get("pipeline"):
        return _build_pipelined(cfg)
    nc = bacc.Bacc("TRN2", target_bir_lowering=False, debug=False)

    xg = nc.dram_tensor("xg", [GPC, X, L], F32R, kind="ExternalInput").ap()
    # W1T packed per quad of groups; each partition reads one contiguous
    # 4*7*32*4B run. w1m[gq, p, j, c, z] = W1[4gq+j][z, row(p, c)] where
    # row depends on x_layout (see _marshal).
    w1m = nc.dram_tensor(
        "w1m", [GPC // 4, P, 4, NCH, Z], F32R, kind="ExternalInput"
    ).ap()
    w2t = nc.dram_tensor("w2t", [Z, GPC * C], F32R, kind="ExternalInput").ap()
    b1c = nc.dram_tensor("b1c", [Z, GPC], F32, kind="ExternalInput").ap()
    btc = nc.dram_tensor("btc", [Z, GPC], F32, kind="ExternalInput").ap()
    b2c = nc.dram_tensor("b2c", [C, GPC], F32, kind="ExternalInput").ap()
    out = nc.dram_tensor("out", [GPC, C, L], F32, kind="ExternalOutput").ap()

    with tile.TileContext(nc) as tc, ExitStack() as ctx:
        consts = ctx.enter_context(tc.tile_pool(name="consts", bufs=1))
        xpool = ctx.enter_context(tc.tile_pool(name="x", bufs=cfg["x_bufs"]))
        wpool = ctx.enter_context(tc.tile_pool(name="w1", bufs=3))
        spool = ctx.enter_context(tc.tile_pool(name="act", bufs=cfg["s_bufs"]))
        hps = ctx.enter_context(
            tc.tile_pool(name="hps", bufs=cfg["h_bufs"], space="PSUM")
        )
        ops = ctx.enter_context(tc.tile_pool(name="ops", bufs=2, space="PSUM"))
        tps = ctx.enter_context(tc.tile_pool(name="tps", bufs=2, space="PSUM"))

        ce = _eng(nc, cfg["const_engine"])
        we = _eng(nc, cfg["w_engine"])
        oe = _eng(nc, cfg["out_engine"])

        # --- constants / per-group scalars ---
        w2tt = consts.tile([Z, GPC * C], F32R, name="w2tt")
        ce.dma_start(w2tt[:], w2t)
        b1t = consts.tile([Z, GPC], F32, name="b1t")
        ce.dma_start(b1t[:], b1c)
        btt = consts.tile([Z, GPC], F32, name="btt")
        ce.dma_start(btt[:], btc)
        b2t = consts.tile([C, GPC], F32, name="b2t")
        ce.dma_start(b2t[:], b2c)
        ones_k = consts.tile([C, 1], F32, name="ones_k")
        nc.vector.memset(ones_k[:], 1.0)
        ones_m = consts.tile([1, C], F32, name="ones_m")
        nc.vector.memset(ones_m[:], 1.0)

        # sp = softplus(beta) = ln(1 + exp(beta)); halves for tanh-sigmoid
        spe = consts.tile([Z, GPC], F32, name="spe")
        nc.scalar.activation(spe[:], btt[:], mybir.ActivationFunctionType.Exp)
        spe1 = consts.tile([Z, GPC], F32, name="spe1")
        nc.vector.tensor_scalar_add(spe1[:], spe[:], 1.0)
        spt = consts.tile([Z, GPC], F32, name="spt")
        nc.scalar.activation(spt[:], spe1[:], mybir.ActivationFunctionType.Ln)
        sph = consts.tile([Z, GPC], F32, name="sph")
        nc.vector.tensor_scalar_mul(sph[:], spt[:], 0.5)
        spb1h = consts.tile([Z, GPC], F32, name="spb1h")
        nc.vector.tensor_mul(spb1h[:], sph[:], b1t[:])

        xt2 = None
        for g in range(GPC):
            gq, jq = divmod(g, 4)
            xe = _eng(nc, cfg["x_engines"][g % len(cfg["x_engines"])])
            if cfg["x_pair"]:
                # one [P, 2*7*L] tile per pair of groups; group g%2==i owns
                # free columns [i*NCH*L, (i+1)*NCH*L) logically remapped below
                if g % 2 == 0:
                    xt2 = xpool.tile([P, 2 * NCH * L], F32R, tag="xt", name=f"xt{g}")
                    xe.dma_start(
                        xt2[:, : 12 * L].rearrange("p (i c l) -> p i c l", i=2, c=6),
                        xg[g : g + 2, : 6 * P].rearrange("i (c p) l -> p i c l", p=P),
                    )
                    xe.dma_start(
                        xt2[:KLAST, 12 * L :].rearrange("p (i l) -> p i l", i=2),
                        xg[g : g + 2, 6 * P :].rearrange("i r l -> r i l"),
                    )
                i = g % 2
                xt = xt2[:, i * 6 * L : (i + 1) * 6 * L]
                xlast = xt2[:, (12 + i) * L : (13 + i) * L]
            elif cfg["x_layout"] == "interleave":
                # chunk c = rows 128c..128c+128; 2KB runs across partitions
                xt = xpool.tile([P, NCH * L], F32R, tag="xt", name=f"xt{g}")
                xlast = xt[:, 6 * L :]
                if cfg["x_split"]:
                    nc.sync.dma_start(
                        xt[:, : 3 * L].rearrange("p (c l) -> p c l", c=3),
                        xg[g, : 3 * P].rearrange("(c p) l -> p c l", p=P),
                    )
                    nc.scalar.dma_start(
                        xt[:, 3 * L : 6 * L].rearrange("p (c l) -> p c l", c=3),
                        xg[g, 3 * P : 6 * P].rearrange("(c p) l -> p c l", p=P),
                    )
                else:
                    xe.dma_start(
                        xt[:, : 6 * L].rearrange("p (c l) -> p c l", c=6),
                        xg[g, : 6 * P].rearrange("(c p) l -> p c l", p=P),
                    )
                xe.dma_start(xt[:KLAST, 6 * L :], xg[g, 6 * P :])
            else:
                # one contiguous run per partition: p<16 -> rows 7p..7p+7,
                # p>=16 -> rows 112+6(p-16)..+6
                xt = xpool.tile([P, NCH * L], F32R, tag="xt", name=f"xt{g}")
                xlast = xt[:, 6 * L :]
                xe.dma_start(
                    xt[:16, :].rearrange("p (c l) -> p c l", c=NCH),
                    xg[g, : 7 * 16].rearrange("(p c) l -> p c l", p=16),
                )
                xe.dma_start(
                    xt[16:, : 6 * L].rearrange("p (c l) -> p c l", c=6),
                    xg[g, 7 * 16 : X].rearrange("(p c) l -> p c l", p=112),
                )
            # --- W1T for a quad of 4 groups, one DMA every 4th group ---
            if jq == 0:
                wt = wpool.tile([P, 4 * NCH * Z], F32R, tag="wt", name=f"wt{g}")
                we.dma_start(
                    wt[:].rearrange("p (j c z) -> p j c z", j=4, c=NCH),
                    w1m[gq],
                )

            # --- h = W1 @ x ---
            h = hps.tile([Z, L], F32, tag="h", name=f"h{g}")
            for c in range(NCH):
                kk = P if c < 6 else KLAST
                rhs = (
                    xt[:, c * L : (c + 1) * L] if c < 6 else xlast[:KLAST, :]
                )
                nc.tensor.matmul(
                    h[:],
                    wt[:kk, (jq * NCH + c) * Z : (jq * NCH + c + 1) * Z],
                    rhs,
                    start=(c == 0),
                    stop=(c == NCH - 1),
                )

            # --- GroupSwish: ((h+b1)*0.5) * (1 + tanh(sp*(h+b1)/2)) ---
            t = spool.tile([Z, L], F32, tag="t", name=f"t{g}")
            nc.scalar.activation(
                t[:],
                h[:],
                mybir.ActivationFunctionType.Tanh,
                bias=spb1h[:, g : g + 1],
                scale=sph[:, g : g + 1],
            )
            u = spool.tile([Z, L], F32, tag="u", name=f"u{g}")
            nc.vector.tensor_scalar(
                u[:],
                h[:],
                b1t[:, g : g + 1],
                0.5,
                op0=mybir.AluOpType.add,
                op1=mybir.AluOpType.mult,
            )
            swish = spool.tile([Z, L], F32R, tag="swish", name=f"sw{g}")
            nc.vector.scalar_tensor_tensor(
                swish[:],
                t[:],
                1.0,
                u[:],
                op0=mybir.AluOpType.add,
                op1=mybir.AluOpType.mult,
            )

            # --- o = (W2/1.1) @ swish ---
            o = ops.tile([C, L], F32, tag="o", name=f"o{g}")
            nc.tensor.matmul(
                o[:], w2tt[:, g * C : (g + 1) * C], swish[:], start=True, stop=True
            )

            # --- softmax over [C, L] (no max subtraction) ---
            expo = spool.tile([C, L], F32, tag="expo", name=f"e{g}")
            esum = spool.tile([C, 1], F32, tag="esum", name=f"es{g}")
            nc.scalar.activation(
                expo[:],
                o[:],
                mybir.ActivationFunctionType.Exp,
                bias=b2t[:, g : g + 1],
                scale=1.0,
                accum_out=esum[:],
            )
            tot = tps.tile([1, 1], F32, tag="tb", name=f"tot{g}")
            nc.tensor.matmul(tot[:], ones_k[:], esum[:], start=True, stop=True)
            inv = spool.tile([1, 1], F32, tag="inv", name=f"inv{g}")
            nc.vector.reciprocal(inv[:], tot[:])
            bc = tps.tile([C, 1], F32, tag="tb", name=f"bc{g}")
            nc.tensor.matmul(bc[:], ones_m[:], inv[:], start=True, stop=True)
            invc = spool.tile([C, 1], F32, tag="invc", name=f"ic{g}")
            nc.vector.tensor_copy(invc[:], bc[:])
            res = spool.tile([C, L], F32, tag="res", name=f"r{g}")
            nc.vector.tensor_scalar_mul(res[:], expo[:], invc[:])

            oe.dma_start(out[g], res[:])

    nc.compile()
    return nc


def _build_pipelined(cfg):
    """Software-pipelined emission: the PE stream per quad q is
    [28x W1(q)] [4x W2(q-1)] [tot4(q-2), bc4(q-2)] so every cross-engine
    dependency (swish from DVE, exp sums from ACT, reciprocal from DVE) has
    a full quad of slack before the PE needs it."""
    nc = bacc.Bacc("TRN2", target_bir_lowering=False, debug=False)
    NQ = GPC // 4

    xg = nc.dram_tensor("xg", [GPC, X, L], F32R, kind="ExternalInput").ap()
    w1m = nc.dram_tensor(
        "w1m", [NQ, P, 4, NCH, Z], F32R, kind="ExternalInput"
    ).ap()
    w2t = nc.dram_tensor("w2t", [Z, GPC * C], F32R, kind="ExternalInput").ap()
    b1c = nc.dram_tensor("b1c", [Z, GPC], F32, kind="ExternalInput").ap()
    btc = nc.dram_tensor("btc", [Z, GPC], F32, kind="ExternalInput").ap()
    b2c = nc.dram_tensor("b2c", [C, GPC], F32, kind="ExternalInput").ap()
    out = nc.dram_tensor("out", [GPC, C, L], F32, kind="ExternalOutput").ap()

    with tile.TileContext(nc) as tc, ExitStack() as ctx:
        consts = ctx.enter_context(tc.tile_pool(name="consts", bufs=1))
        xpool = ctx.enter_context(tc.tile_pool(name="x", bufs=cfg["x_bufs"]))
        wpool = ctx.enter_context(tc.tile_pool(name="w1", bufs=3))
        spool = ctx.enter_context(tc.tile_pool(name="act", bufs=cfg["s_bufs"]))
        dpool = ctx.enter_context(tc.tile_pool(name="deep", bufs=10))
        e4pool = ctx.enter_context(tc.tile_pool(name="e4", bufs=3))
        hps = ctx.enter_context(
            tc.tile_pool(name="hps", bufs=cfg["h_bufs"], space="PSUM")
        )
        ops = ctx.enter_context(tc.tile_pool(name="ops", bufs=2, space="PSUM"))
        tps = ctx.enter_context(tc.tile_pool(name="tps", bufs=2, space="PSUM"))

        oe = _eng(nc, cfg["out_engine"])
        ce = _eng(nc, cfg["const_engine"])
        we = _eng(nc, cfg["w_engine"])

        w2tt = consts.tile([Z, GPC * C], F32R, name="w2tt")
        ce.dma_start(w2tt[:], w2t)
        b1t = consts.tile([Z, GPC], F32, name="b1t")
        ce.dma_start(b1t[:], b1c)
        btt = consts.tile([Z, GPC], F32, name="btt")
        ce.dma_start(btt[:], btc)
        b2t = consts.tile([C, GPC], F32, name="b2t")
        ce.dma_start(b2t[:], b2c)
        ones_k = consts.tile([C, 1], F32, name="ones_k")
        nc.vector.memset(ones_k[:], 1.0)
        ones_m = consts.tile([1, C], F32, name="ones_m")
        nc.vector.memset(ones_m[:], 1.0)

        spe = consts.tile([Z, GPC], F32, name="spe")
        nc.scalar.activation(spe[:], btt[:], mybir.ActivationFunctionType.Exp)
        spe1 = consts.tile([Z, GPC], F32, name="spe1")
        nc.vector.tensor_scalar_add(spe1[:], spe[:], 1.0)
        spt = consts.tile([Z, GPC], F32, name="spt")
        nc.scalar.activation(spt[:], spe1[:], mybir.ActivationFunctionType.Ln)
        sph = consts.tile([Z, GPC], F32, name="sph")
        nc.vector.tensor_scalar_mul(sph[:], spt[:], 0.5)
        spb1h = consts.tile([Z, GPC], F32, name="spb1h")
        nc.vector.tensor_mul(spb1h[:], sph[:], b1t[:])

        swishes = {}  # g -> tile
        expos = {}  # g -> tile
        esums = {}  # q -> [C, 4] tile
        n_x = len(cfg["x_engines"])

        def stage1(q):
            """x/w loads, W1 matmuls, swish for quad q."""
            wt = wpool.tile([P, 4 * NCH * Z], F32R, tag="wt", name=f"wt{q}")
            we.dma_start(
                wt[:].rearrange("p (j c z) -> p j c z", j=4, c=NCH), w1m[q]
            )
            for j in range(4):
                g = 4 * q + j
                xe = _eng(nc, cfg["x_engines"][g % n_x])
                xt = xpool.tile([P, NCH * L], F32R, tag="xt", name=f"xt{g}")
                if cfg["x_split"]:
                    nc.sync.dma_start(
                        xt[:, : 3 * L].rearrange("p (c l) -> p c l", c=3),
                        xg[g, : 3 * P].rearrange("(c p) l -> p c l", p=P),
                    )
                    nc.scalar.dma_start(
                        xt[:, 3 * L : 6 * L].rearrange("p (c l) -> p c l", c=3),
                        xg[g, 3 * P : 6 * P].rearrange("(c p) l -> p c l", p=P),
                    )
                else:
                    xe.dma_start(
                        xt[:, : 6 * L].rearrange("p (c l) -> p c l", c=6),
                        xg[g, : 6 * P].rearrange("(c p) l -> p c l", p=P),
                    )
                xe.dma_start(xt[:KLAST, 6 * L :], xg[g, 6 * P :])

                h = hps.tile([Z, L], F32, tag="h", name=f"h{g}")
                for c in range(NCH):
                    kk = P if c < 6 else KLAST
                    nc.tensor.matmul(
                        h[:],
                        wt[:kk, (j * NCH + c) * Z : (j * NCH + c + 1) * Z],
                        xt[:kk, c * L : (c + 1) * L],
                        start=(c == 0),
                        stop=(c == NCH - 1),
                    )
                t = spool.tile([Z, L], F32, tag="t", name=f"t{g}")
                nc.scalar.activation(
                    t[:],
                    h[:],
                    mybir.ActivationFunctionType.Tanh,
                    bias=spb1h[:, g : g + 1],
                    scale=sph[:, g : g + 1],
                )
                u = spool.tile([Z, L], F32, tag="u", name=f"u{g}")
                nc.vector.tensor_scalar(
                    u[:],
                    h[:],
                    b1t[:, g : g + 1],
                    0.5,
                    op0=mybir.AluOpType.add,
                    op1=mybir.AluOpType.mult,
                )
                sw = dpool.tile([Z, L], F32R, tag="swish", name=f"sw{g}")
                nc.vector.scalar_tensor_tensor(
                    sw[:],
                    t[:],
                    1.0,
                    u[:],
                    op0=mybir.AluOpType.add,
                    op1=mybir.AluOpType.mult,
                )
                swishes[g] = sw

        def stage2(q):
            """W2 matmuls + exp for quad q (emitted one quad later)."""
            esum4 = e4pool.tile([C, 4], F32, tag="esum4", name=f"es4_{q}")
            esums[q] = esum4
            for j in range(4):
                g = 4 * q + j
                o = ops.tile([C, L], F32, tag="o", name=f"o{g}")
                nc.tensor.matmul(
                    o[:],
                    w2tt[:, g * C : (g + 1) * C],
                    swishes.pop(g)[:],
                    start=True,
                    stop=True,
                )
                expo = dpool.tile([C, L], F32, tag="expo", name=f"e{g}")
                nc.scalar.activation(
                    expo[:],
                    o[:],
                    mybir.ActivationFunctionType.Exp,
                    bias=b2t[:, g : g + 1],
                    scale=1.0,
                    accum_out=esum4[:, j : j + 1],
                )
                expos[g] = expo

        def stage3(q):
            """Normalization + store for quad q (emitted two quads later)."""
            esum4 = esums.pop(q)
            tot4 = tps.tile([1, 4], F32, tag="tb", name=f"tot{q}")
            nc.tensor.matmul(tot4[:], ones_k[:], esum4[:], start=True, stop=True)
            inv4 = spool.tile([1, 4], F32, tag="inv", name=f"inv{q}")
            nc.vector.reciprocal(inv4[:], tot4[:])
            bc4 = tps.tile([C, 4], F32, tag="tb", name=f"bc{q}")
            nc.tensor.matmul(bc4[:], ones_m[:], inv4[:], start=True, stop=True)
            invc4 = spool.tile([C, 4], F32, tag="invc", name=f"ic{q}")
            nc.vector.tensor_copy(invc4[:], bc4[:])
            for j in range(4):
                g = 4 * q + j
                res = spool.tile([C, L], F32, tag="res", name=f"r{g}")
                nc.vector.tensor_scalar_mul(
                    res[:], expos.pop(g)[:], invc4[:, j : j + 1]
                )
                oe.dma_start(out[g], res[:])

        for q in range(NQ):
            stage1(q)
            if q >= 1:
                stage2(q - 1)
            if q >= 2:
                stage3(q - 2)
        stage2(NQ - 1)
        stage3(NQ - 2)
        stage3(NQ - 1)

    nc.compile()
    return nc


def _marshal(x, W1, b1, beta, W2, b2, cfg=DEFAULT_CFG):
    """Full inputs -> list of per-core input dicts."""
    xg = np.ascontiguousarray(x, dtype=np.float32).reshape(B, X, L)
    w1T = W1.astype(np.float32, copy=False).transpose(0, 2, 1)  # [B, X, Z]
    w1m = np.zeros((B // 4, P, 4, NCH, Z), np.float32)
    if cfg["x_layout"] == "interleave":
        # w1m[gq, p, j, c, z] = W1T[4gq+j, 128c+p, z]
        main = w1T[:, : 6 * P].reshape(B // 4, 4, 6, P, Z)
        w1m[:, :, :, :6] = main.transpose(0, 3, 1, 2, 4)
        left = w1T[:, 6 * P :].reshape(B // 4, 4, KLAST, Z)
        w1m[:, :KLAST, :, 6] = left.transpose(0, 2, 1, 3)
    else:
        # row(p, c) = 7p+c for p<16, 112+6(p-16)+c for p>=16
        lo = w1T[:, : 7 * 16].reshape(B // 4, 4, 16, NCH, Z)
        hi = w1T[:, 7 * 16 :].reshape(B // 4, 4, 112, 6, Z)
        w1m[:, :16] = lo.transpose(0, 2, 1, 3, 4)
        w1m[:, 16:, :, :6] = hi.transpose(0, 2, 1, 3, 4)
    w2s = (W2.astype(np.float32, copy=False) * np.float32(1.0 / 1.1)).transpose(
        0, 2, 1
    )  # [B, Z, C]

    in_maps = []
    for core in range(NCORE):
        s = slice(core * GPC, (core + 1) * GPC)
        sq = slice(core * GPC // 4, (core + 1) * GPC // 4)
        in_maps.append(
            {
                "xg": xg[s],
                "w1m": w1m[sq],
                # [Z, GPC*C]: w2t[z, g*C+c] = W2[g0+g, c, z] / 1.1
                "w2t": np.ascontiguousarray(
                    w2s[s].transpose(1, 0, 2).reshape(Z, GPC * C)
                ),
                "b1c": np.ascontiguousarray(b1[s].astype(np.float32).T),
                "btc": np.ascontiguousarray(
                    np.broadcast_to(beta[s].astype(np.float32), (Z, GPC))
                ),
                "b2c": np.ascontiguousarray(b2[s].astype(np.float32).T),
            }
        )
    return in_maps


def _run(in_maps, cfg=DEFAULT_CFG, trace=False, tmpdir=None):
    key = str(sorted(cfg.items()))
    if key not in _CACHE:
        _CACHE[key] = _build(cfg)
    return run_bass_kernel_spmd(
        _CACHE[key],
        in_maps,
        core_ids=list(range(NCORE)),
        trace=trace,
        tmpdir=tmpdir,
    )


_LAST = {}


def kernel(x, W1, b1, beta, W2, b2):
    in_maps = _marshal(x, W1, b1, beta, W2, b2)
    trace = bool(os.environ.get("KERNEL_TRACE"))
    r = _run(in_maps, trace=trace, tmpdir=os.environ.get("KERNEL_TRACE_DIR"))
    _LAST["results"] = r
    outs = [r.results[c]["out"].reshape(GPC, C * L) for c in range(NCORE)]
    return np.concatenate(outs, axis=0)



# revision 16
# speedup vs baseline: 2.6316x; 2.6316x over previous
"""Grouped per-sample MLP (conv1d groups=B) + GroupSwish + softmax, on 8 NeuronCores.

Data-parallel over the group/batch axis B=256: 32 groups per core,
processed as 8 quads of 4 groups stacked on the partition axis.

Per group g: h = W1[g] @ x[g] + b1[g]; GroupSwish; o = W2[g] @ h + b2[g];
softmax over the flattened [C*L] logits.

v1 design (bf16 + quad stacking):
  - x and W1 cast to bf16 host-side (error budget: gate 2e-2, bf16 gives ~1e-3).
    x pre-transposed host-side so each SBUF partition loads ONE contiguous
    6KB run (line-rate DMA descriptors instead of 2KB ones).
  - W1 matmuls: per group 6 K-chunks of 128; the K=16 remainder of all 4
    groups of a quad is folded into ONE block-diagonal matmul (64 partitions),
    25 matmuls/quad instead of 28. PSUM matmul writes only allow base
    partition {0,32,64}, so group j=3 uses a 64-wide lhsT [zeros|W1] at
    base 64; its c=0 matmul goes first (start=True zeroes 64:128) and the
    j=2 chain accumulates with start=False on top of the zeros.
  - h for 4 groups lives in one [128, 512] PSUM tile (partitions 32j..32j+31),
    so GroupSwish runs as ONE tanh-ACT + 2 DVE ops per quad (4x fewer
    instructions than per-group).
  - W2 is block-diagonalized host-side to [128, 40] per quad: ONE matmul
    produces all 4 groups' logits [40, 512]; exp+accum, block-sum and
    broadcast via tiny mask matmuls, 1/1.1 folded into W2, softplus(beta)
    and its products computed host-side into a const blob.
  - GroupSwish via tanh (the only ACT table with both tanh and exp):
    (h+b1)*sigmoid(sp*(h+b1)) = ((h+b1)*0.5) * (1 + tanh(sp*(h+b1)/2)).
  - Softmax without max-subtraction (logits are O(1)).
"""

import os
import numpy as np
import ml_dtypes
from contextlib import ExitStack

import concourse.mybir as mybir
import concourse.tile as tile
from concourse import bacc
from concourse.bass_utils import run_bass_kernel_spmd

B, X, Z, C, L = 256, 784, 32, 10, 512
NCORE = 8
GPC = B // NCORE  # 32 groups per core
NQ = GPC // 4  # 8 quads per core
P = 128
NCH = 6  # dense K-chunks of 128; remainder 16 handled block-diagonally
KREM = X - NCH * P  # 16
F32 = mybir.dt.float32
BF16 = mybir.dt.bfloat16
BF16NP = ml_dtypes.bfloat16

# const-blob column layout (f32, [128, CB_COLS])
CB_SPH = 0  # softplus(beta)/2, stacked [32j+z, q]
CB_SPB1 = NQ  # sph * b1
CB_B1 = 2 * NQ  # b1
CB_B2 = 3 * NQ  # b2 stacked [10j+c, q] (rows 0..40)
CB_BLK = 4 * NQ  # ones_blk [40, 4]: 1 if row//10 == col
CB_BCM = 4 * NQ + 4  # bcmask [4, 40]: 1 if col//10 == row
CB_COLS = 4 * NQ + 4 + 40

DEFAULT_CFG = dict(
    x_engines=("sync", "scalar", "sync", "scalar"),  # per group-in-quad
    w_engine="gpsimd",
    out_engine="gpsimd",
    const_engine="gpsimd",
    x_bufs=8,
    h_bufs=3,
    s_bufs=3,
    rem_combined=True,  # one block-diag remainder matmul per quad
    debug=False,  # dump h and o per quad to DRAM
)

_CACHE: dict = {}


def _eng(nc, name):
    return getattr(nc, name)


def _build(cfg=DEFAULT_CFG):
    nc = bacc.Bacc("TRN2", target_bir_lowering=False, debug=False)

    xm = nc.dram_tensor("xm", [GPC, P, NCH * L], BF16, kind="ExternalInput").ap()
    xr = nc.dram_tensor("xr", [NQ, 4 * KREM, L], BF16, kind="ExternalInput").ap()
    # per chunk c: cols [0:32) j0, [32:64) j1, [64:96) j2, [96:160) = [0|W1_j3]
    w1m = nc.dram_tensor(
        "w1m", [NQ, P, NCH * 5 * Z], BF16, kind="ExternalInput"
    ).ap()
    w1r = nc.dram_tensor("w1r", [NQ, 4 * KREM, P], BF16, kind="ExternalInput").ap()
    w2b = nc.dram_tensor("w2b", [P, NQ * 4 * C], BF16, kind="ExternalInput").ap()
    cb = nc.dram_tensor("cb", [P, CB_COLS], F32, kind="ExternalInput").ap()
    out = nc.dram_tensor("out", [GPC, C, L], F32, kind="ExternalOutput").ap()
    if cfg.get("debug"):
        hdbg = nc.dram_tensor("hdbg", [NQ, P, L], F32, kind="ExternalOutput").ap()
        odbg = nc.dram_tensor("odbg", [NQ, 4 * C, L], F32, kind="ExternalOutput").ap()
        edbg = nc.dram_tensor("edbg", [NQ, 4 * C, L], F32, kind="ExternalOutput").ap()
        sdbg = nc.dram_tensor("sdbg", [NQ, 4 * C, 3], F32, kind="ExternalOutput").ap()

    with tile.TileContext(nc) as tc, ExitStack() as ctx:
        consts = ctx.enter_context(tc.tile_pool(name="consts", bufs=1))
        xpool = ctx.enter_context(tc.tile_pool(name="x", bufs=cfg["x_bufs"]))
        wpool = ctx.enter_context(tc.tile_pool(name="w1", bufs=3))
        wrpool = ctx.enter_context(tc.tile_pool(name="w1r", bufs=3))
        spool = ctx.enter_context(tc.tile_pool(name="act", bufs=cfg["s_bufs"]))
        hps = ctx.enter_context(
            tc.tile_pool(name="hps", bufs=cfg["h_bufs"], space="PSUM")
        )
        ops = ctx.enter_context(tc.tile_pool(name="ops", bufs=2, space="PSUM"))
        tps = ctx.enter_context(tc.tile_pool(name="tps", bufs=1, space="PSUM"))

        ce = _eng(nc, cfg["const_engine"])
        we = _eng(nc, cfg["w_engine"])
        oe = _eng(nc, cfg["out_engine"])
        n_x = len(cfg["x_engines"])

        w2t = consts.tile([P, NQ * 4 * C], BF16, name="w2t")
        ce.dma_start(w2t[:], w2b)
        cbt = consts.tile([P, CB_COLS], F32, name="cbt")
        ce.dma_start(cbt[:], cb)

        for q in range(NQ):
            wt = wpool.tile([P, NCH * 5 * Z], BF16, tag="wt", name=f"wt{q}")
            we.dma_start(wt[:], w1m[q])
            wr = wrpool.tile([4 * KREM, P], BF16, tag="wr", name=f"wr{q}")
            we.dma_start(wr[:], w1r[q])
            xrt = xpool.tile([4 * KREM, L], BF16, tag="xr", name=f"xr{q}")
            we.dma_start(xrt[:], xr[q])
            xts = []
            for j in range(4):
                g = 4 * q + j
                xe = _eng(nc, cfg["x_engines"][j % n_x])
                xt = xpool.tile([P, NCH * L], BF16, tag="xt", name=f"xt{g}")
                xe.dma_start(xt[:], xm[g])
                xts.append(xt)

            # --- h[32j:32j+32] = W1[g] @ x[g] for the quad ---
            # start=True clears has_written for the WHOLE bank then overwrites;
            # start=False accumulates where the bit is set. So the block-diag
            # K=16 remainder (which writes all 128 partitions) goes FIRST with
            # start=True; every dense matmul then accumulates, order-free.
            # j=3 uses a 64-wide [zeros|W1] lhsT at base 64 (PSUM matmul
            # writes only allow base partition 0/32/64); the zeros add 0.
            h = hps.tile([P, L], F32, tag="h", name=f"h{q}")
            nc.tensor.matmul(
                h[:], wr[:], xrt[:], start=True, stop=False, skip_group_check=True
            )
            for j in range(4):
                for c in range(NCH):
                    if j == 3:
                        lhs = wt[:, (c * 5 + 3) * Z : (c * 5 + 5) * Z]
                        dst = h[64:128, :]
                    else:
                        lhs = wt[:, (c * 5 + j) * Z : (c * 5 + j + 1) * Z]
                        dst = h[32 * j : 32 * (j + 1), :]
                    nc.tensor.matmul(
                        dst,
                        lhs,
                        xts[j][:, c * L : (c + 1) * L],
                        start=False,
                        stop=(j == 3 and c == NCH - 1),
                        skip_group_check=True,
                    )

            if cfg.get("debug"):
                hcp = spool.tile([P, L], F32, tag="hcp", name=f"hcp{q}")
                nc.vector.tensor_copy(hcp[:], h[:])
                oe.dma_start(hdbg[q], hcp[:])

            # --- GroupSwish: ((h+b1)*0.5) * (1 + tanh(sp*(h+b1)/2)) ---
            t = spool.tile([P, L], F32, tag="t", name=f"t{q}")
            nc.scalar.activation(
                t[:],
                h[:],
                mybir.ActivationFunctionType.Tanh,
                bias=cbt[:, CB_SPB1 + q : CB_SPB1 + q + 1],
                scale=cbt[:, CB_SPH + q : CB_SPH + q + 1],
            )
            u = spool.tile([P, L], F32, tag="u", name=f"u{q}")
            nc.vector.tensor_scalar(
                u[:],
                h[:],
                cbt[:, CB_B1 + q : CB_B1 + q + 1],
                0.5,
                op0=mybir.AluOpType.add,
                op1=mybir.AluOpType.mult,
            )
            sw = spool.tile([P, L], BF16, tag="sw", name=f"sw{q}")
            nc.vector.scalar_tensor_tensor(
                sw[:],
                t[:],
                1.0,
                u[:],
                op0=mybir.AluOpType.add,
                op1=mybir.AluOpType.mult,
            )

            # --- o = blockdiag(W2/1.1) @ swish : all 4 groups in one matmul ---
            o = ops.tile([4 * C, L], F32, tag="o", name=f"o{q}")
            nc.tensor.matmul(
                o[:],
                w2t[:, q * 4 * C : (q + 1) * 4 * C],
                sw[:],
                start=True,
                stop=True,
            )

            # --- softmax over [C, L] per group (no max subtraction) ---
            expo = spool.tile([4 * C, L], F32, tag="expo", name=f"e{q}")
            esum = spool.tile([4 * C, 1], F32, tag="esum", name=f"es{q}")
            nc.scalar.activation(
                expo[:],
                o[:],
                mybir.ActivationFunctionType.Exp,
                bias=cbt[: 4 * C, CB_B2 + q : CB_B2 + q + 1],
                scale=1.0,
                accum_out=esum[:],
            )
            if cfg.get("debug"):
                ocp = spool.tile([4 * C, L], F32, tag="ocp", name=f"ocp{q}")
                nc.vector.tensor_copy(ocp[:], o[:])
                oe.dma_start(odbg[q], ocp[:])

            tot = tps.tile([4, 1], F32, tag="tot", name=f"tot{q}")
            nc.tensor.matmul(
                tot[:],
                cbt[: 4 * C, CB_BLK : CB_BLK + 4],
                esum[:],
                start=True,
                stop=True,
            )
            inv = spool.tile([4, 1], F32, tag="inv", name=f"inv{q}")
            nc.vector.reciprocal(inv[:], tot[:])
            bc = tps.tile([4 * C, 1], F32, tag="bc", name=f"bc{q}")
            nc.tensor.matmul(
                bc[:],
                cbt[:4, CB_BCM : CB_BCM + 4 * C],
                inv[:],
                start=True,
                stop=True,
            )
            invc = spool.tile([4 * C, 1], F32, tag="invc", name=f"ic{q}")
            nc.vector.tensor_copy(invc[:], bc[:])
            if cfg.get("debug"):
                scp = spool.tile([4 * C, 3], F32, tag="scp", name=f"scp{q}")
                nc.vector.tensor_copy(scp[:, 0:1], esum[:])
                nc.vector.tensor_copy(scp[:, 1:2], invc[:])
                nc.vector.memset(scp[:, 2:3], 0.0)
                nc.vector.tensor_copy(scp[:4, 2:3], tot[:])
                oe.dma_start(sdbg[q], scp[:])
                oe.dma_start(edbg[q], expo[:])
            res = spool.tile([4 * C, L], F32, tag="res", name=f"r{q}")
            nc.vector.tensor_scalar_mul(res[:], expo[:], invc[:, 0:1])

            oe.dma_start(
                out[4 * q : 4 * (q + 1)].rearrange("j c l -> (j c) l"),
                res[:],
            )

    nc.compile()
    return nc


def _marshal(x, W1, b1, beta, W2, b2, cfg=DEFAULT_CFG):
    """Full inputs -> list of per-core input dicts (all heavy reshapes here)."""
    xg = np.ascontiguousarray(x, dtype=np.float32).reshape(B, X, L)
    # xm[g, p, c*L+l] = x[g, 128c+p, l]
    xmain = (
        xg[:, : NCH * P]
        .reshape(B, NCH, P, L)
        .transpose(0, 2, 1, 3)
        .astype(BF16NP)
        .reshape(B, P, NCH * L)
    )
    # xrem[gq, 16j+r, l] = x[4gq+j, 768+r, l]
    xrem = xg[:, NCH * P :].astype(BF16NP).reshape(B // 4, 4 * KREM, L)

    w1T = W1.astype(np.float32, copy=False).transpose(0, 2, 1)  # [B, X, Z]
    # w1m[gq, p, c, 5*Z]: cols [0:32) j0, [32:64) j1, [64:96) j2,
    # [96:128) zeros, [128:160) j3  (j3 slot is the 64-wide padded lhsT)
    w1d = (
        w1T[:, : NCH * P]
        .reshape(B // 4, 4, NCH, P, Z)
        .transpose(0, 3, 2, 1, 4)
        .astype(BF16NP)
    )  # [B//4, P, NCH, 4, Z]
    w1m = np.zeros((B // 4, P, NCH, 5 * Z), BF16NP)
    w1m[..., : 3 * Z] = w1d[..., :3, :].reshape(B // 4, P, NCH, 3 * Z)
    w1m[..., 4 * Z :] = w1d[..., 3, :]
    w1m = w1m.reshape(B // 4, P, NCH * 5 * Z)
    # w1r[gq, 16j+r, 32j+z] = W1T[4gq+j, 768+r, z], else 0 (block diagonal)
    w1r = np.zeros((B // 4, 4 * KREM, P), BF16NP)
    w1T4 = w1T.reshape(B // 4, 4, X, Z)
    for j in range(4):
        w1r[:, KREM * j : KREM * (j + 1), Z * j : Z * (j + 1)] = w1T4[
            :, j, NCH * P :
        ].astype(BF16NP)

    # w2b[gq, 32j+z, 10j+c] = W2[4gq+j, c, z]/1.1, else 0 (block diagonal)
    w2s = (W2.astype(np.float32, copy=False) * np.float32(1.0 / 1.1)).transpose(
        0, 2, 1
    )  # [B, Z, C]
    w2blk = np.zeros((B // 4, P, 4 * C), BF16NP)
    w2s4 = w2s.reshape(B // 4, 4, Z, C)
    for j in range(4):
        w2blk[:, Z * j : Z * (j + 1), C * j : C * (j + 1)] = w2s4[:, j].astype(
            BF16NP
        )

    # const blob per core [128, CB_COLS] f32
    b1f = b1.astype(np.float32, copy=False)
    b2f = b2.astype(np.float32, copy=False)
    sp = np.log1p(np.exp(beta.astype(np.float64))).astype(np.float32)  # softplus
    blk = np.zeros((P, 4), np.float32)
    bcm = np.zeros((P, 4 * C), np.float32)
    for j in range(4):
        blk[C * j : C * (j + 1), j] = 1.0
        bcm[j, C * j : C * (j + 1)] = 1.0

    in_maps = []
    for core in range(NCORE):
        s = slice(core * GPC, (core + 1) * GPC)
        sq = slice(core * NQ, (core + 1) * NQ)
        cbc = np.zeros((P, CB_COLS), np.float32)
        # stacked [32j+z, q] views for this core's quads
        b1c = b1f[s].reshape(NQ, 4 * Z).T  # [128, NQ]
        spc = np.repeat(sp[s].reshape(NQ, 4), Z, axis=1).T * 0.5  # [128, NQ]
        cbc[:, CB_SPH : CB_SPH + NQ] = spc
        cbc[:, CB_SPB1 : CB_SPB1 + NQ] = spc * b1c
        cbc[:, CB_B1 : CB_B1 + NQ] = b1c
        cbc[: 4 * C, CB_B2 : CB_B2 + NQ] = b2f[s].reshape(NQ, 4 * C).T
        cbc[:, CB_BLK : CB_BLK + 4] = blk
        cbc[:, CB_BCM : CB_BCM + 4 * C] = bcm
        in_maps.append(
            {
                "xm": xmain[s],
                "xr": xrem[sq],
                "w1m": w1m[sq],
                "w1r": w1r[sq],
                "w2b": np.ascontiguousarray(
                    w2blk[sq].transpose(1, 0, 2).reshape(P, NQ * 4 * C)
                ),
                "cb": cbc,
            }
        )
    return in_maps


def _run(in_maps, cfg=DEFAULT_CFG, trace=False, tmpdir=None):
    key = str(sorted(cfg.items()))
    if key not in _CACHE:
        _CACHE[key] = _build(cfg)
    return run_bass_kernel_spmd(
        _CACHE[key],
        in_maps,
        core_ids=list(range(NCORE)),
        trace=trace,
        tmpdir=tmpdir,
    )


_LAST = {}


def kernel(x, W1, b1, beta, W2, b2):
    in_maps = _marshal(x, W1, b1, beta, W2, b2)
    trace = bool(os.environ.get("KERNEL_TRACE"))
    r = _run(in_maps, trace=trace, tmpdir=os.environ.get("KERNEL_TRACE_DIR"))
    _LAST["results"] = r
    outs = [r.results[c]["out"].reshape(GPC, C * L) for c in range(NCORE)]
    return np.concatenate(outs, axis=0)


# revision 20
# speedup vs baseline: 3.0363x; 1.1538x over previous
"""Grouped per-sample MLP (conv1d groups=B) + GroupSwish + softmax, on 8 NeuronCores.

Data-parallel over the group/batch axis B=256: 32 groups per core,
processed as 8 quads of 4 groups stacked on the partition axis.

Per group g: h = W1[g] @ x[g] + b1[g]; GroupSwish; o = W2[g] @ h + b2[g];
softmax over the flattened [C*L] logits.

v2 design (fp8 DoubleRow + quad stacking):
  - x and W1 cast to fp8 e4m3 host-side (numpy-simulated end-to-end rel err
    8.8e-3 vs the 2e-2 gate). W1 scaled by 16 to stay in e4m3 normal range;
    the 1/16 is folded into the activation scales. x pre-transposed host-side
    so each SBUF partition loads contiguous runs (line-rate DMA descriptors).
  - W1 matmuls in fp8 DoubleRow perf mode: chunk pairs (K=256 per matmul,
    0.5 cycles/row) -> 3 matmuls per group, 13 per quad total.
  - PSUM start=True clears has_written for the WHOLE bank; to stay
    order-robust there is exactly ONE start=True matmul per quad: the
    block-diagonal K=16 remainder, which writes all 128 partitions first.
    Everything else accumulates (start=False), so scheduler order is free.
  - h for 4 groups lives in one [128, 512] PSUM tile (partitions 32j..32j+31),
    so GroupSwish runs as ONE tanh-ACT + 2 DVE ops per quad. PSUM matmul
    writes only allow base partition {0,32,64}, so group j=3 uses a 64-wide
    [zeros|W1] lhsT at base 64.
  - W2 is block-diagonalized host-side to [128, 40] bf16 per quad: ONE matmul
    produces all 4 groups' logits [40, 512]; exp+accum, block-sum and
    broadcast via tiny mask matmuls; 1/1.1 folded into W2; softplus(beta)
    and its products computed host-side into a const blob.
  - GroupSwish via tanh (the only ACT table with both tanh and exp):
    (h+b1)*sigmoid(sp*(h+b1)) = ((h+b1)*0.5) * (1 + tanh(sp*(h+b1)/2)).
  - Softmax without max-subtraction (logits are O(1)).
"""

import os
import numpy as np
import ml_dtypes
from contextlib import ExitStack

import concourse.mybir as mybir
import concourse.tile as tile
from concourse import bacc
from concourse.bass_utils import run_bass_kernel_spmd

B, X, Z, C, L = 256, 784, 32, 10, 512
NCORE = 8
GPC = B // NCORE  # 32 groups per core
NQ = GPC // 4  # 8 quads per core
P = 128
NCH = 6  # dense K-chunks of 128 (3 DoubleRow pairs); remainder 16 block-diag
KREM = X - NCH * P  # 16
W1SC = 16.0  # host-side W1 scale (fp8 normal range); folded into act scales
F32 = mybir.dt.float32
BF16 = mybir.dt.bfloat16
FP8 = mybir.dt.float8e4
BF16NP = ml_dtypes.bfloat16
FP8NP = mybir.dt.np(mybir.dt.float8e4)

# DoubleRow matmuls may only write PSUM at base partition 0 (probed:
# M32@0/M64@0/M128@0 pass walrus, anything at base 32/64 fails ISA checks).
# So lhsT is zero-padded per group so every DR write lands at base 0:
#   j0: [W1|0]  64-wide -> h[0:64]     j1: [0|W1]  64-wide -> h[0:64]
#   j2: [0,0,W1,0] 128-wide -> h[:]    j3: [0,0,0,W1] 128-wide -> h[:]
# per-pair block layout (pair i covers chunks 2i, 2i+1), offsets in elements:
# [j0: 2*64 @0][j1: 2*64 @128][j2: 2*128 @256][j3: 2*128 @512] = 768/pair
PAIRW = 768
J_OFF = (0, 128, 256, 512)
J_W = (64, 64, 128, 128)
WCOLS = 3 * PAIRW  # 2304

# const-blob column layout (f32, [128, CB_COLS])
CB_SPH = 0  # softplus(beta)/2/W1SC (tanh scale on h*W1SC), stacked [32j+z, q]
CB_SPB1 = NQ  # (softplus(beta)/2) * b1 (tanh bias)
CB_B1 = 2 * NQ  # W1SC * b1 (u path: (h16 + 16 b1) * (0.5/16))
CB_B2 = 3 * NQ  # b2 stacked [10j+c, q] (rows 0..40)
CB_BLK = 4 * NQ  # ones_blk [40, 4]: 1 if row//10 == col
CB_BCM = 4 * NQ + 4  # bcmask [4, 40]: 1 if col//10 == row
CB_COLS = 4 * NQ + 4 + 40

DEFAULT_CFG = dict(
    w_engine="gpsimd",
    out_engine="gpsimd",
    const_engine="gpsimd",
    x_bufs=10,
    h_bufs=3,
    s_bufs=3,
    x_split=True,  # chunks 0-3 and 4-5 as separate DMAs for finer overlap
    debug=False,  # dump h and o per quad to DRAM
)

_CACHE: dict = {}


def _eng(nc, name):
    return getattr(nc, name)


def _build(cfg=DEFAULT_CFG):
    nc = bacc.Bacc("TRN2", target_bir_lowering=False, debug=False)

    xm = nc.dram_tensor("xm", [GPC, P, NCH * L], FP8, kind="ExternalInput").ap()
    xr = nc.dram_tensor("xr", [NQ, 4 * KREM, L], FP8, kind="ExternalInput").ap()
    w1m = nc.dram_tensor("w1m", [NQ, P, WCOLS], FP8, kind="ExternalInput").ap()
    w1r = nc.dram_tensor("w1r", [NQ, 4 * KREM, P], FP8, kind="ExternalInput").ap()
    w2b = nc.dram_tensor("w2b", [P, NQ * 4 * C], BF16, kind="ExternalInput").ap()
    cb = nc.dram_tensor("cb", [P, CB_COLS], F32, kind="ExternalInput").ap()
    out = nc.dram_tensor("out", [GPC, C, L], F32, kind="ExternalOutput").ap()
    if cfg.get("debug"):
        hdbg = nc.dram_tensor("hdbg", [NQ, P, L], F32, kind="ExternalOutput").ap()
        odbg = nc.dram_tensor("odbg", [NQ, 4 * C, L], F32, kind="ExternalOutput").ap()

    with tile.TileContext(nc) as tc, ExitStack() as ctx:
        consts = ctx.enter_context(tc.tile_pool(name="consts", bufs=1))
        xpool = ctx.enter_context(tc.tile_pool(name="x", bufs=cfg["x_bufs"]))
        wpool = ctx.enter_context(tc.tile_pool(name="w1", bufs=3))
        wrpool = ctx.enter_context(tc.tile_pool(name="w1r", bufs=3))
        spool = ctx.enter_context(tc.tile_pool(name="act", bufs=cfg["s_bufs"]))
        hps = ctx.enter_context(
            tc.tile_pool(name="hps", bufs=cfg["h_bufs"], space="PSUM")
        )
        ops = ctx.enter_context(tc.tile_pool(name="ops", bufs=2, space="PSUM"))
        tps = ctx.enter_context(tc.tile_pool(name="tps", bufs=1, space="PSUM"))

        ce = _eng(nc, cfg["const_engine"])
        we = _eng(nc, cfg["w_engine"])
        oe = _eng(nc, cfg["out_engine"])

        w2t = consts.tile([P, NQ * 4 * C], BF16, name="w2t")
        ce.dma_start(w2t[:], w2b)
        cbt = consts.tile([P, CB_COLS], F32, name="cbt")
        ce.dma_start(cbt[:], cb)

        for q in range(NQ):
            wt = wpool.tile([P, WCOLS], FP8, tag="wt", name=f"wt{q}")
            we.dma_start(wt[:], w1m[q])
            wr = wrpool.tile([4 * KREM, P], FP8, tag="wr", name=f"wr{q}")
            nc.scalar.dma_start(wr[:], w1r[q])
            xrt = xpool.tile([4 * KREM, L], FP8, tag="xr", name=f"xr{q}")
            nc.sync.dma_start(xrt[:], xr[q])
            xts = []
            for j in range(4):
                g = 4 * q + j
                xt = xpool.tile([P, NCH * L], FP8, tag="xt", name=f"xt{g}")
                e1, e2 = (nc.sync, nc.scalar) if j % 2 == 0 else (nc.scalar, nc.sync)
                if cfg["x_split"]:
                    e1.dma_start(xt[:, : 4 * L], xm[g, :, : 4 * L])
                    e2.dma_start(xt[:, 4 * L :], xm[g, :, 4 * L :])
                else:
                    e1.dma_start(xt[:], xm[g])
                xts.append(xt)

            # --- h[32j:32j+32] = W1SC * W1[g] @ x[g] for the quad ---
            # ONE start=True matmul per quad (the K=16 remainder, which
            # writes all 128 partitions); all DoubleRow matmuls accumulate.
            h = hps.tile([P, L], F32, tag="h", name=f"h{q}")
            nc.tensor.matmul(
                h[:], wr[:], xrt[:], start=True, stop=False, skip_group_check=True
            )
            for j in range(4):
                for i in range(3):
                    off, w = J_OFF[j], J_W[j]
                    lhs = wt[:, i * PAIRW + off : i * PAIRW + off + 2 * w]
                    lhs = lhs.rearrange("p (c m) -> p c m", c=2)
                    dst = h[0:w, :]
                    rhs = xts[j][:, 2 * i * L : 2 * (i + 1) * L].rearrange(
                        "p (c l) -> p c l", c=2
                    )
                    nc.tensor.matmul(
                        dst,
                        lhs,
                        rhs,
                        start=False,
                        stop=(j == 3 and i == 2),
                        skip_group_check=True,
                        perf_mode=mybir.MatmulPerfMode.DoubleRow,
                    )

            if cfg.get("debug"):
                hcp = spool.tile([P, L], F32, tag="hcp", name=f"hcp{q}")
                nc.vector.tensor_copy(hcp[:], h[:])
                oe.dma_start(hdbg[q], hcp[:])

            # --- GroupSwish: ((h+b1)*0.5) * (1 + tanh(sp*(h+b1)/2)) ---
            # device h is W1SC*(W1@x); scales in cb fold the 1/W1SC back in
            t = spool.tile([P, L], F32, tag="t", name=f"t{q}")
            nc.scalar.activation(
                t[:],
                h[:],
                mybir.ActivationFunctionType.Tanh,
                bias=cbt[:, CB_SPB1 + q : CB_SPB1 + q + 1],
                scale=cbt[:, CB_SPH + q : CB_SPH + q + 1],
            )
            u = spool.tile([P, L], F32, tag="u", name=f"u{q}")
            nc.vector.tensor_scalar(
                u[:],
                h[:],
                cbt[:, CB_B1 + q : CB_B1 + q + 1],
                0.5 / W1SC,
                op0=mybir.AluOpType.add,
                op1=mybir.AluOpType.mult,
            )
            sw = spool.tile([P, L], BF16, tag="sw", name=f"sw{q}")
            nc.vector.scalar_tensor_tensor(
                sw[:],
                t[:],
                1.0,
                u[:],
                op0=mybir.AluOpType.add,
                op1=mybir.AluOpType.mult,
            )

            # --- o = blockdiag(W2/1.1) @ swish : all 4 groups in one matmul ---
            o = ops.tile([4 * C, L], F32, tag="o", name=f"o{q}")
            nc.tensor.matmul(
                o[:],
                w2t[:, q * 4 * C : (q + 1) * 4 * C],
                sw[:],
                start=True,
                stop=True,
            )
            if cfg.get("debug"):
                ocp = spool.tile([4 * C, L], F32, tag="ocp", name=f"ocp{q}")
                nc.vector.tensor_copy(ocp[:], o[:])
                oe.dma_start(odbg[q], ocp[:])

            # --- softmax over [C, L] per group (no max subtraction) ---
            expo = spool.tile([4 * C, L], F32, tag="expo", name=f"e{q}")
            esum = spool.tile([4 * C, 1], F32, tag="esum", name=f"es{q}")
            nc.scalar.activation(
                expo[:],
                o[:],
                mybir.ActivationFunctionType.Exp,
                bias=cbt[: 4 * C, CB_B2 + q : CB_B2 + q + 1],
                scale=1.0,
                accum_out=esum[:],
            )
            tot = tps.tile([4, 1], F32, tag="tot", name=f"tot{q}")
            nc.tensor.matmul(
                tot[:],
                cbt[: 4 * C, CB_BLK : CB_BLK + 4],
                esum[:],
                start=True,
                stop=True,
            )
            inv = spool.tile([4, 1], F32, tag="inv", name=f"inv{q}")
            nc.vector.reciprocal(inv[:], tot[:])
            bc = tps.tile([4 * C, 1], F32, tag="bc", name=f"bc{q}")
            nc.tensor.matmul(
                bc[:],
                cbt[:4, CB_BCM : CB_BCM + 4 * C],
                inv[:],
                start=True,
                stop=True,
            )
            invc = spool.tile([4 * C, 1], F32, tag="invc", name=f"ic{q}")
            nc.vector.tensor_copy(invc[:], bc[:])
            res = spool.tile([4 * C, L], F32, tag="res", name=f"r{q}")
            nc.vector.tensor_scalar_mul(res[:], expo[:], invc[:, 0:1])

            oe.dma_start(
                out[4 * q : 4 * (q + 1)].rearrange("j c l -> (j c) l"),
                res[:],
            )

    nc.compile()
    return nc


def _marshal(x, W1, b1, beta, W2, b2, cfg=DEFAULT_CFG):
    """Full inputs -> list of per-core input dicts (all heavy reshapes here)."""
    xg = np.ascontiguousarray(x, dtype=np.float32).reshape(B, X, L)
    # xm[g, p, c*L+l] = x[g, 128c+p, l]
    xmain = (
        xg[:, : NCH * P]
        .reshape(B, NCH, P, L)
        .transpose(0, 2, 1, 3)
        .astype(FP8NP)
        .reshape(B, P, NCH * L)
    )
    # xrem[gq, 16j+r, l] = x[4gq+j, 768+r, l]
    xrem = xg[:, NCH * P :].astype(FP8NP).reshape(B // 4, 4 * KREM, L)

    w1s = W1.astype(np.float32, copy=False) * np.float32(W1SC)
    w1T = w1s.transpose(0, 2, 1)  # [B, X, Z]
    w1ck = w1T[:, : NCH * P].reshape(B // 4, 4, NCH, P, Z)  # [gq, j, c, p, z]
    w1m = np.zeros((B // 4, P, WCOLS), np.float32)
    for i in range(3):
        for cc in range(2):
            c = 2 * i + cc
            for j in range(4):
                base = i * PAIRW + J_OFF[j] + cc * J_W[j] + 32 * j
                w1m[:, :, base : base + Z] = w1ck[:, j, c]
    w1m = w1m.astype(FP8NP)
    # w1r[gq, 16j+r, 32j+z] = W1SC*W1T[4gq+j, 768+r, z], else 0 (block diag)
    w1r = np.zeros((B // 4, 4 * KREM, P), FP8NP)
    w1T4 = w1T.reshape(B // 4, 4, X, Z)
    for j in range(4):
        w1r[:, KREM * j : KREM * (j + 1), Z * j : Z * (j + 1)] = w1T4[
            :, j, NCH * P :
        ].astype(FP8NP)

    # w2b[gq, 32j+z, 10j+c] = W2[4gq+j, c, z]/1.1, else 0 (block diagonal)
    w2s = (W2.astype(np.float32, copy=False) * np.float32(1.0 / 1.1)).transpose(
        0, 2, 1
    )  # [B, Z, C]
    w2blk = np.zeros((B // 4, P, 4 * C), BF16NP)
    w2s4 = w2s.reshape(B // 4, 4, Z, C)
    for j in range(4):
        w2blk[:, Z * j : Z * (j + 1), C * j : C * (j + 1)] = w2s4[:, j].astype(
            BF16NP
        )

    # const blob per core [128, CB_COLS] f32
    b1f = b1.astype(np.float32, copy=False)
    b2f = b2.astype(np.float32, copy=False)
    sp = np.log1p(np.exp(beta.astype(np.float64))).astype(np.float32)  # softplus
    blk = np.zeros((P, 4), np.float32)
    bcm = np.zeros((P, 4 * C), np.float32)
    for j in range(4):
        blk[C * j : C * (j + 1), j] = 1.0
        bcm[j, C * j : C * (j + 1)] = 1.0

    in_maps = []
    for core in range(NCORE):
        s = slice(core * GPC, (core + 1) * GPC)
        sq = slice(core * NQ, (core + 1) * NQ)
        cbc = np.zeros((P, CB_COLS), np.float32)
        # stacked [32j+z, q] views for this core's quads
        b1c = b1f[s].reshape(NQ, 4 * Z).T  # [128, NQ]
        spc = np.repeat(sp[s].reshape(NQ, 4), Z, axis=1).T * 0.5  # [128, NQ]
        cbc[:, CB_SPH : CB_SPH + NQ] = spc / np.float32(W1SC)
        cbc[:, CB_SPB1 : CB_SPB1 + NQ] = spc * b1c
        cbc[:, CB_B1 : CB_B1 + NQ] = b1c * np.float32(W1SC)
        cbc[: 4 * C, CB_B2 : CB_B2 + NQ] = b2f[s].reshape(NQ, 4 * C).T
        cbc[:, CB_BLK : CB_BLK + 4] = blk
        cbc[:, CB_BCM : CB_BCM + 4 * C] = bcm
        in_maps.append(
            {
                "xm": xmain[s],
                "xr": xrem[sq],
                "w1m": w1m[sq],
                "w1r": w1r[sq],
                "w2b": np.ascontiguousarray(
                    w2blk[sq].transpose(1, 0, 2).reshape(P, NQ * 4 * C)
                ),
                "cb": cbc,
            }
        )
    return in_maps


def _run(in_maps, cfg=DEFAULT_CFG, trace=False, tmpdir=None):
    key = str(sorted(cfg.items()))
    if key not in _CACHE:
        _CACHE[key] = _build(cfg)
    return run_bass_kernel_spmd(
        _CACHE[key],
        in_maps,
        core_ids=list(range(NCORE)),
        trace=trace,
        tmpdir=tmpdir,
    )


_LAST = {}


def kernel(x, W1, b1, beta, W2, b2):
    in_maps = _marshal(x, W1, b1, beta, W2, b2)
    trace = bool(os.environ.get("KERNEL_TRACE"))
    r = _run(in_maps, trace=trace, tmpdir=os.environ.get("KERNEL_TRACE_DIR"))
    _LAST["results"] = r
    outs = [r.results[c]["out"].reshape(GPC, C * L) for c in range(NCORE)]
    return np.concatenate(outs, axis=0)


# revision 26
# speedup vs baseline: 3.3559x; 1.1052x over previous
"""Grouped per-sample MLP (conv1d groups=B) + GroupSwish + softmax, on 8 NeuronCores.

Data-parallel over the group/batch axis B=256: 32 groups per core,
processed as 8 quads of 4 groups stacked on the partition axis.

Per group g: h = W1[g] @ x[g] + b1[g]; GroupSwish; o = W2[g] @ h + b2[g];
softmax over the flattened [C*L] logits.

v2 design (fp8 DoubleRow + quad stacking):
  - x and W1 cast to fp8 e4m3 host-side (numpy-simulated end-to-end rel err
    8.8e-3 vs the 2e-2 gate). W1 scaled by 16 to stay in e4m3 normal range;
    the 1/16 is folded into the activation scales. x pre-transposed host-side
    so each SBUF partition loads contiguous runs (line-rate DMA descriptors).
  - W1 matmuls in fp8 DoubleRow perf mode: chunk pairs (K=256 per matmul,
    0.5 cycles/row) -> 3 matmuls per group, 13 per quad total.
  - PSUM start=True clears has_written for the WHOLE bank; to stay
    order-robust there is exactly ONE start=True matmul per quad: the
    block-diagonal K=16 remainder, which writes all 128 partitions first.
    Everything else accumulates (start=False), so scheduler order is free.
  - h for 4 groups lives in one [128, 512] PSUM tile (partitions 32j..32j+31),
    so GroupSwish runs as ONE tanh-ACT + 2 DVE ops per quad. PSUM matmul
    writes only allow base partition {0,32,64}, so group j=3 uses a 64-wide
    [zeros|W1] lhsT at base 64.
  - W2 is block-diagonalized host-side to [128, 40] bf16 per quad: ONE matmul
    produces all 4 groups' logits [40, 512]; exp+accum, block-sum and
    broadcast via tiny mask matmuls; 1/1.1 folded into W2; softplus(beta)
    and its products computed host-side into a const blob.
  - GroupSwish via tanh (the only ACT table with both tanh and exp):
    (h+b1)*sigmoid(sp*(h+b1)) = ((h+b1)*0.5) * (1 + tanh(sp*(h+b1)/2)).
  - Softmax without max-subtraction (logits are O(1)).
"""

import os
import numpy as np
import ml_dtypes
from contextlib import ExitStack

import concourse.mybir as mybir
import concourse.tile as tile
from concourse import bacc
from concourse.bass_utils import run_bass_kernel_spmd

B, X, Z, C, L = 256, 784, 32, 10, 512
NCORE = 8
GPC = B // NCORE  # 32 groups per core
NQ = GPC // 4  # 8 quads per core
P = 128
NCH = 6  # dense K-chunks of 128 (3 DoubleRow pairs); remainder 16 block-diag
KREM = X - NCH * P  # 16
W1SC = 16.0  # host-side W1 scale (fp8 normal range); folded into act scales
F32 = mybir.dt.float32
BF16 = mybir.dt.bfloat16
FP8 = mybir.dt.float8e4
BF16NP = ml_dtypes.bfloat16
FP8NP = mybir.dt.np(mybir.dt.float8e4)

# DoubleRow matmuls may only write PSUM at base partition 0 (probed:
# M32@0/M64@0/M128@0 pass walrus, anything at base 32/64 fails ISA checks).
# So lhsT is zero-padded per group so every DR write lands at base 0:
#   j0: [W1|0]  64-wide -> h[0:64]     j1: [0|W1]  64-wide -> h[0:64]
#   j2: [0,0,W1,0] 128-wide -> h[:]    j3: [0,0,0,W1] 128-wide -> h[:]
# per-pair block layout (pair i covers chunks 2i, 2i+1), offsets in elements:
# [j0: 2*64 @0][j1: 2*64 @128][j2: 2*128 @256][j3: 2*128 @512] = 768/pair
PAIRW = 768
J_OFF = (0, 128, 256, 512)
J_W = (64, 64, 128, 128)
WCOLS = 3 * PAIRW  # 2304

# const-blob column layout (f32, [128, CB_COLS])
CB_SPH = 0  # softplus(beta)/2/W1SC (tanh scale on h*W1SC), stacked [32j+z, q]
CB_SPB1 = NQ  # (softplus(beta)/2) * b1 (tanh bias)
CB_B1 = 2 * NQ  # W1SC * b1 (u path: (h16 + 16 b1) * (0.5/16))
CB_B2 = 3 * NQ  # b2 stacked [10j+c, q] (rows 0..40)
CB_BLK = 4 * NQ  # ones_blk [40, 4]: 1 if row//10 == col
CB_BCM = 4 * NQ + 4  # bcmask [4, 40]: 1 if col//10 == row
CB_COLS = 4 * NQ + 4 + 40

DEFAULT_CFG = dict(
    w_engine="gpsimd",
    out_engine="gpsimd",
    const_engine="gpsimd",
    x_bufs=6,
    h_bufs=3,
    s_bufs=3,
    x_split=True,  # chunks 0-3 and 4-5 as separate DMAs for finer overlap
    debug=False,  # dump h and o per quad to DRAM
)

_CACHE: dict = {}


def _eng(nc, name):
    return getattr(nc, name)


def _build(cfg=DEFAULT_CFG):
    nc = bacc.Bacc("TRN2", target_bir_lowering=False, debug=False)

    # two groups packed per row so each partition loads one 6KB run
    xm = nc.dram_tensor(
        "xm", [GPC // 2, P, 2 * NCH * L], FP8, kind="ExternalInput"
    ).ap()
    xr = nc.dram_tensor("xr", [NQ, 4 * KREM, L], FP8, kind="ExternalInput").ap()
    w1m = nc.dram_tensor("w1m", [NQ, P, WCOLS], FP8, kind="ExternalInput").ap()
    w1r = nc.dram_tensor("w1r", [NQ, 4 * KREM, P], FP8, kind="ExternalInput").ap()
    w2b = nc.dram_tensor("w2b", [P, NQ * 4 * C], BF16, kind="ExternalInput").ap()
    cb = nc.dram_tensor("cb", [P, CB_COLS], F32, kind="ExternalInput").ap()
    out = nc.dram_tensor("out", [GPC, C, L], F32, kind="ExternalOutput").ap()
    if cfg.get("debug"):
        hdbg = nc.dram_tensor("hdbg", [NQ, P, L], F32, kind="ExternalOutput").ap()
        odbg = nc.dram_tensor("odbg", [NQ, 4 * C, L], F32, kind="ExternalOutput").ap()

    with tile.TileContext(nc) as tc, ExitStack() as ctx:
        consts = ctx.enter_context(tc.tile_pool(name="consts", bufs=1))
        xpool = ctx.enter_context(tc.tile_pool(name="x", bufs=cfg["x_bufs"]))
        wpool = ctx.enter_context(tc.tile_pool(name="w1", bufs=3))
        wrpool = ctx.enter_context(tc.tile_pool(name="w1r", bufs=3))
        spool = ctx.enter_context(tc.tile_pool(name="act", bufs=cfg["s_bufs"]))
        hps = ctx.enter_context(
            tc.tile_pool(name="hps", bufs=cfg["h_bufs"], space="PSUM")
        )
        ops = ctx.enter_context(tc.tile_pool(name="ops", bufs=2, space="PSUM"))
        tps = ctx.enter_context(tc.tile_pool(name="tps", bufs=1, space="PSUM"))

        ce = _eng(nc, cfg["const_engine"])
        we = _eng(nc, cfg["w_engine"])
        oe = _eng(nc, cfg["out_engine"])

        w2t = consts.tile([P, NQ * 4 * C], BF16, name="w2t")
        ce.dma_start(w2t[:], w2b)
        cbt = consts.tile([P, CB_COLS], F32, name="cbt")
        ce.dma_start(cbt[:], cb)

        for q in range(NQ):
            wt = wpool.tile([P, WCOLS], FP8, tag="wt", name=f"wt{q}")
            we.dma_start(wt[:], w1m[q])
            wr = wrpool.tile([4 * KREM, P], FP8, tag="wr", name=f"wr{q}")
            nc.scalar.dma_start(wr[:], w1r[q])
            xrt = xpool.tile([4 * KREM, L], FP8, tag="xr", name=f"xr{q}")
            nc.sync.dma_start(xrt[:], xr[q])
            xts = []
            for pp in range(2):
                gp = 2 * q + pp
                xt = xpool.tile([P, 2 * NCH * L], FP8, tag="xt", name=f"xt{gp}")
                xe = nc.sync if pp == 0 else nc.scalar
                xe.dma_start(xt[:], xm[gp])
                xts.append(xt)

            # --- h[32j:32j+32] = W1SC * W1[g] @ x[g] for the quad ---
            # ONE start=True matmul per quad (the K=16 remainder, which
            # writes all 128 partitions); all DoubleRow matmuls accumulate.
            h = hps.tile([P, L], F32, tag="h", name=f"h{q}")
            nc.tensor.matmul(
                h[:], wr[:], xrt[:], start=True, stop=False, skip_group_check=True
            )
            for j in range(4):
                for i in range(3):
                    off, w = J_OFF[j], J_W[j]
                    lhs = wt[:, i * PAIRW + off : i * PAIRW + off + 2 * w]
                    lhs = lhs.rearrange("p (c m) -> p c m", c=2)
                    dst = h[0:w, :]
                    xb = (j % 2) * NCH * L
                    rhs = xts[j // 2][
                        :, xb + 2 * i * L : xb + 2 * (i + 1) * L
                    ].rearrange("p (c l) -> p c l", c=2)
                    nc.tensor.matmul(
                        dst,
                        lhs,
                        rhs,
                        start=False,
                        stop=(j == 3 and i == 2),
                        skip_group_check=True,
                        perf_mode=mybir.MatmulPerfMode.DoubleRow,
                    )

            if cfg.get("debug"):
                hcp = spool.tile([P, L], F32, tag="hcp", name=f"hcp{q}")
                nc.vector.tensor_copy(hcp[:], h[:])
                oe.dma_start(hdbg[q], hcp[:])

            # --- GroupSwish: ((h+b1)*0.5) * (1 + tanh(sp*(h+b1)/2)) ---
            # device h is W1SC*(W1@x); scales in cb fold the 1/W1SC back in
            t = spool.tile([P, L], F32, tag="t", name=f"t{q}")
            nc.scalar.activation(
                t[:],
                h[:],
                mybir.ActivationFunctionType.Tanh,
                bias=cbt[:, CB_SPB1 + q : CB_SPB1 + q + 1],
                scale=cbt[:, CB_SPH + q : CB_SPH + q + 1],
            )
            u = spool.tile([P, L], F32, tag="u", name=f"u{q}")
            nc.vector.tensor_scalar(
                u[:],
                h[:],
                cbt[:, CB_B1 + q : CB_B1 + q + 1],
                0.5 / W1SC,
                op0=mybir.AluOpType.add,
                op1=mybir.AluOpType.mult,
            )
            sw = spool.tile([P, L], BF16, tag="sw", name=f"sw{q}")
            nc.vector.scalar_tensor_tensor(
                sw[:],
                t[:],
                1.0,
                u[:],
                op0=mybir.AluOpType.add,
                op1=mybir.AluOpType.mult,
            )

            # --- o = blockdiag(W2/1.1) @ swish : all 4 groups in one matmul ---
            o = ops.tile([4 * C, L], F32, tag="o", name=f"o{q}")
            nc.tensor.matmul(
                o[:],
                w2t[:, q * 4 * C : (q + 1) * 4 * C],
                sw[:],
                start=True,
                stop=True,
            )
            if cfg.get("debug"):
                ocp = spool.tile([4 * C, L], F32, tag="ocp", name=f"ocp{q}")
                nc.vector.tensor_copy(ocp[:], o[:])
                oe.dma_start(odbg[q], ocp[:])

            # --- softmax over [C, L] per group (no max subtraction) ---
            expo = spool.tile([4 * C, L], F32, tag="expo", name=f"e{q}")
            esum = spool.tile([4 * C, 1], F32, tag="esum", name=f"es{q}")
            nc.scalar.activation(
                expo[:],
                o[:],
                mybir.ActivationFunctionType.Exp,
                bias=cbt[: 4 * C, CB_B2 + q : CB_B2 + q + 1],
                scale=1.0,
                accum_out=esum[:],
            )
            tot = tps.tile([4, 1], F32, tag="tot", name=f"tot{q}")
            nc.tensor.matmul(
                tot[:],
                cbt[: 4 * C, CB_BLK : CB_BLK + 4],
                esum[:],
                start=True,
                stop=True,
            )
            inv = spool.tile([4, 1], F32, tag="inv", name=f"inv{q}")
            nc.vector.reciprocal(inv[:], tot[:])
            bc = tps.tile([4 * C, 1], F32, tag="bc", name=f"bc{q}")
            nc.tensor.matmul(
                bc[:],
                cbt[:4, CB_BCM : CB_BCM + 4 * C],
                inv[:],
                start=True,
                stop=True,
            )
            invc = spool.tile([4 * C, 1], F32, tag="invc", name=f"ic{q}")
            nc.vector.tensor_copy(invc[:], bc[:])
            res = spool.tile([4 * C, L], F32, tag="res", name=f"r{q}")
            nc.vector.tensor_scalar_mul(res[:], expo[:], invc[:, 0:1])

            oe.dma_start(
                out[4 * q : 4 * (q + 1)].rearrange("j c l -> (j c) l"),
                res[:],
            )

    nc.compile()
    return nc


def _marshal(x, W1, b1, beta, W2, b2, cfg=DEFAULT_CFG):
    """Full inputs -> list of per-core input dicts (all heavy reshapes here)."""
    xg = np.ascontiguousarray(x, dtype=np.float32).reshape(B, X, L)
    # xm[gp, p, jj*NCH*L + c*L + l] = x[2gp+jj, 128c+p, l]
    xmain = (
        xg[:, : NCH * P]
        .reshape(B // 2, 2, NCH, P, L)
        .transpose(0, 3, 1, 2, 4)
        .astype(FP8NP)
        .reshape(B // 2, P, 2 * NCH * L)
    )
    # xrem[gq, 16j+r, l] = x[4gq+j, 768+r, l]
    xrem = xg[:, NCH * P :].astype(FP8NP).reshape(B // 4, 4 * KREM, L)

    w1s = W1.astype(np.float32, copy=False) * np.float32(W1SC)
    w1T = w1s.transpose(0, 2, 1)  # [B, X, Z]
    w1ck = w1T[:, : NCH * P].reshape(B // 4, 4, NCH, P, Z)  # [gq, j, c, p, z]
    w1m = np.zeros((B // 4, P, WCOLS), np.float32)
    for i in range(3):
        for cc in range(2):
            c = 2 * i + cc
            for j in range(4):
                base = i * PAIRW + J_OFF[j] + cc * J_W[j] + 32 * j
                w1m[:, :, base : base + Z] = w1ck[:, j, c]
    w1m = w1m.astype(FP8NP)
    # w1r[gq, 16j+r, 32j+z] = W1SC*W1T[4gq+j, 768+r, z], else 0 (block diag)
    w1r = np.zeros((B // 4, 4 * KREM, P), FP8NP)
    w1T4 = w1T.reshape(B // 4, 4, X, Z)
    for j in range(4):
        w1r[:, KREM * j : KREM * (j + 1), Z * j : Z * (j + 1)] = w1T4[
            :, j, NCH * P :
        ].astype(FP8NP)

    # w2b[gq, 32j+z, 10j+c] = W2[4gq+j, c, z]/1.1, else 0 (block diagonal)
    w2s = (W2.astype(np.float32, copy=False) * np.float32(1.0 / 1.1)).transpose(
        0, 2, 1
    )  # [B, Z, C]
    w2blk = np.zeros((B // 4, P, 4 * C), BF16NP)
    w2s4 = w2s.reshape(B // 4, 4, Z, C)
    for j in range(4):
        w2blk[:, Z * j : Z * (j + 1), C * j : C * (j + 1)] = w2s4[:, j].astype(
            BF16NP
        )

    # const blob per core [128, CB_COLS] f32
    b1f = b1.astype(np.float32, copy=False)
    b2f = b2.astype(np.float32, copy=False)
    sp = np.log1p(np.exp(beta.astype(np.float64))).astype(np.float32)  # softplus
    blk = np.zeros((P, 4), np.float32)
    bcm = np.zeros((P, 4 * C), np.float32)
    for j in range(4):
        blk[C * j : C * (j + 1), j] = 1.0
        bcm[j, C * j : C * (j + 1)] = 1.0

    in_maps = []
    for core in range(NCORE):
        s = slice(core * GPC, (core + 1) * GPC)
        sq = slice(core * NQ, (core + 1) * NQ)
        cbc = np.zeros((P, CB_COLS), np.float32)
        # stacked [32j+z, q] views for this core's quads
        b1c = b1f[s].reshape(NQ, 4 * Z).T  # [128, NQ]
        spc = np.repeat(sp[s].reshape(NQ, 4), Z, axis=1).T * 0.5  # [128, NQ]
        cbc[:, CB_SPH : CB_SPH + NQ] = spc / np.float32(W1SC)
        cbc[:, CB_SPB1 : CB_SPB1 + NQ] = spc * b1c
        cbc[:, CB_B1 : CB_B1 + NQ] = b1c * np.float32(W1SC)
        cbc[: 4 * C, CB_B2 : CB_B2 + NQ] = b2f[s].reshape(NQ, 4 * C).T
        cbc[:, CB_BLK : CB_BLK + 4] = blk
        cbc[:, CB_BCM : CB_BCM + 4 * C] = bcm
        sp2 = slice(core * GPC // 2, (core + 1) * GPC // 2)
        in_maps.append(
            {
                "xm": xmain[sp2],
                "xr": xrem[sq],
                "w1m": w1m[sq],
                "w1r": w1r[sq],
                "w2b": np.ascontiguousarray(
                    w2blk[sq].transpose(1, 0, 2).reshape(P, NQ * 4 * C)
                ),
                "cb": cbc,
            }
        )
    return in_maps


def _run(in_maps, cfg=DEFAULT_CFG, trace=False, tmpdir=None):
    key = str(sorted(cfg.items()))
    if key not in _CACHE:
        _CACHE[key] = _build(cfg)
    return run_bass_kernel_spmd(
        _CACHE[key],
        in_maps,
        core_ids=list(range(NCORE)),
        trace=trace,
        tmpdir=tmpdir,
    )


_LAST = {}


def kernel(x, W1, b1, beta, W2, b2):
    in_maps = _marshal(x, W1, b1, beta, W2, b2)
    trace = bool(os.environ.get("KERNEL_TRACE"))
    r = _run(in_maps, trace=trace, tmpdir=os.environ.get("KERNEL_TRACE_DIR"))
    _LAST["results"] = r
    outs = [r.results[c]["out"].reshape(GPC, C * L) for c in range(NCORE)]
    return np.concatenate(outs, axis=0)


# revision 28
# speedup vs baseline: 3.5291x; 1.0516x over previous
"""Grouped per-sample MLP (conv1d groups=B) + GroupSwish + softmax, on 8 NeuronCores.

Data-parallel over the group/batch axis B=256: 32 groups per core,
processed as 8 quads of 4 groups stacked on the partition axis.

Per group g: h = W1[g] @ x[g] + b1[g]; GroupSwish; o = W2[g] @ h + b2[g];
softmax over the flattened [C*L] logits.

v2 design (fp8 DoubleRow + quad stacking):
  - x and W1 cast to fp8 e4m3 host-side (numpy-simulated end-to-end rel err
    8.8e-3 vs the 2e-2 gate). W1 scaled by 16 to stay in e4m3 normal range;
    the 1/16 is folded into the activation scales. x pre-transposed host-side
    so each SBUF partition loads contiguous runs (line-rate DMA descriptors).
  - W1 matmuls in fp8 DoubleRow perf mode: chunk pairs (K=256 per matmul,
    0.5 cycles/row) -> 3 matmuls per group, 13 per quad total.
  - PSUM start=True clears has_written for the WHOLE bank; to stay
    order-robust there is exactly ONE start=True matmul per quad: the
    block-diagonal K=16 remainder, which writes all 128 partitions first.
    Everything else accumulates (start=False), so scheduler order is free.
  - h for 4 groups lives in one [128, 512] PSUM tile (partitions 32j..32j+31),
    so GroupSwish runs as ONE tanh-ACT + 2 DVE ops per quad. PSUM matmul
    writes only allow base partition {0,32,64}, so group j=3 uses a 64-wide
    [zeros|W1] lhsT at base 64.
  - W2 is block-diagonalized host-side to [128, 40] bf16 per quad: ONE matmul
    produces all 4 groups' logits [40, 512]; exp+accum, block-sum and
    broadcast via tiny mask matmuls; 1/1.1 folded into W2; softplus(beta)
    and its products computed host-side into a const blob.
  - GroupSwish via tanh (the only ACT table with both tanh and exp):
    (h+b1)*sigmoid(sp*(h+b1)) = ((h+b1)*0.5) * (1 + tanh(sp*(h+b1)/2)).
  - Softmax without max-subtraction (logits are O(1)).
"""

import os
import numpy as np
import ml_dtypes
from contextlib import ExitStack

import concourse.mybir as mybir
import concourse.tile as tile
from concourse import bacc
from concourse.bass_utils import run_bass_kernel_spmd

B, X, Z, C, L = 256, 784, 32, 10, 512
NCORE = 8
GPC = B // NCORE  # 32 groups per core
NQ = GPC // 4  # 8 quads per core
P = 128
NCH = 6  # dense K-chunks of 128 (3 DoubleRow pairs); remainder 16 block-diag
KREM = X - NCH * P  # 16
W1SC = 16.0  # host-side W1 scale (fp8 normal range); folded into act scales
F32 = mybir.dt.float32
BF16 = mybir.dt.bfloat16
FP8 = mybir.dt.float8e4
BF16NP = ml_dtypes.bfloat16
FP8NP = mybir.dt.np(mybir.dt.float8e4)

# DoubleRow matmuls may only write PSUM at base partition 0 (probed:
# M32@0/M64@0/M128@0 pass walrus, anything at base 32/64 fails ISA checks).
# So lhsT is zero-padded per group so every DR write lands at base 0:
#   j0: [W1|0]  64-wide -> h[0:64]     j1: [0|W1]  64-wide -> h[0:64]
#   j2: [0,0,W1,0] 128-wide -> h[:]    j3: [0,0,0,W1] 128-wide -> h[:]
# per-pair block layout (pair i covers chunks 2i, 2i+1), offsets in elements:
# [j0: 2*64 @0][j1: 2*64 @128][j2: 2*128 @256][j3: 2*128 @512] = 768/pair
PAIRW = 768
J_OFF = (0, 128, 256, 512)
J_W = (64, 64, 128, 128)
WCOLS = 3 * PAIRW  # 2304

# const-blob column layout (f32, [128, CB_COLS])
CB_SPH = 0  # softplus(beta)/2/W1SC (tanh scale on h*W1SC), stacked [32j+z, q]
CB_SPB1 = NQ  # (softplus(beta)/2) * b1 (tanh bias)
CB_B1 = 2 * NQ  # W1SC * b1 (u path: (h16 + 16 b1) * (0.5/16))
CB_B2 = 3 * NQ  # b2 stacked [10j+c, q] (rows 0..40)
CB_BLK = 4 * NQ  # ones_blk [40, 4]: 1 if row//10 == col
CB_BCM = 4 * NQ + 4  # bcmask [4, 40]: 1 if col//10 == row
CB_COLS = 4 * NQ + 4 + 40

DEFAULT_CFG = dict(
    w_engine="gpsimd",
    out_engine="gpsimd",
    const_engine="gpsimd",
    x_bufs=8,
    h_bufs=3,
    s_bufs=3,
    x_split=True,  # chunks 0-3 and 4-5 as separate DMAs for finer overlap
    debug=False,  # dump h and o per quad to DRAM
)

_CACHE: dict = {}


def _eng(nc, name):
    return getattr(nc, name)


def _build(cfg=DEFAULT_CFG):
    nc = bacc.Bacc("TRN2", target_bir_lowering=False, debug=False)

    # two groups packed per row so each partition loads one 6KB run
    xm = nc.dram_tensor(
        "xm", [GPC // 2, P, 2 * NCH * L], FP8, kind="ExternalInput"
    ).ap()
    xr = nc.dram_tensor("xr", [NQ, 4 * KREM, L], FP8, kind="ExternalInput").ap()
    w1m = nc.dram_tensor("w1m", [NQ, P, WCOLS], FP8, kind="ExternalInput").ap()
    w1r = nc.dram_tensor("w1r", [NQ, 4 * KREM, P], FP8, kind="ExternalInput").ap()
    w2b = nc.dram_tensor("w2b", [P, NQ * 4 * C], BF16, kind="ExternalInput").ap()
    cb = nc.dram_tensor("cb", [P, CB_COLS], F32, kind="ExternalInput").ap()
    out = nc.dram_tensor("out", [GPC, C, L], F32, kind="ExternalOutput").ap()
    if cfg.get("debug"):
        hdbg = nc.dram_tensor("hdbg", [NQ, P, L], F32, kind="ExternalOutput").ap()
        odbg = nc.dram_tensor("odbg", [NQ, 4 * C, L], F32, kind="ExternalOutput").ap()

    with tile.TileContext(nc) as tc, ExitStack() as ctx:
        consts = ctx.enter_context(tc.tile_pool(name="consts", bufs=1))
        xpool = ctx.enter_context(tc.tile_pool(name="x", bufs=cfg["x_bufs"]))
        wpool = ctx.enter_context(tc.tile_pool(name="w1", bufs=3))
        wrpool = ctx.enter_context(tc.tile_pool(name="w1r", bufs=3))
        spool = ctx.enter_context(tc.tile_pool(name="act", bufs=cfg["s_bufs"]))
        hps = ctx.enter_context(
            tc.tile_pool(name="hps", bufs=cfg["h_bufs"], space="PSUM")
        )
        ops = ctx.enter_context(tc.tile_pool(name="ops", bufs=2, space="PSUM"))
        tps = ctx.enter_context(tc.tile_pool(name="tps", bufs=1, space="PSUM"))

        ce = _eng(nc, cfg["const_engine"])
        we = _eng(nc, cfg["w_engine"])
        oe = _eng(nc, cfg["out_engine"])

        w2t = consts.tile([P, NQ * 4 * C], BF16, name="w2t")
        ce.dma_start(w2t[:], w2b)
        cbt = consts.tile([P, CB_COLS], F32, name="cbt")
        ce.dma_start(cbt[:], cb)

        for q in range(NQ):
            # weights on the fast HWDGE rings so they never queue behind
            # the out writes on the gpsimd SWDGE FIFO
            w_ring = nc.sync if q % 2 == 0 else nc.scalar
            wt = wpool.tile([P, WCOLS], FP8, tag="wt", name=f"wt{q}")
            w_ring.dma_start(wt[:], w1m[q])
            wr = wrpool.tile([4 * KREM, P], FP8, tag="wr", name=f"wr{q}")
            nc.scalar.dma_start(wr[:], w1r[q])
            xrt = xpool.tile([4 * KREM, L], FP8, tag="xr", name=f"xr{q}")
            nc.sync.dma_start(xrt[:], xr[q])
            xts = []
            for pp in range(2):
                gp = 2 * q + pp
                xt = xpool.tile([P, 2 * NCH * L], FP8, tag="xt", name=f"xt{gp}")
                xe = nc.sync if pp == 0 else nc.scalar
                xe.dma_start(xt[:], xm[gp])
                xts.append(xt)

            # --- h[32j:32j+32] = W1SC * W1[g] @ x[g] for the quad ---
            # ONE start=True matmul per quad (the K=16 remainder, which
            # writes all 128 partitions); all DoubleRow matmuls accumulate.
            h = hps.tile([P, L], F32, tag="h", name=f"h{q}")
            nc.tensor.matmul(
                h[:], wr[:], xrt[:], start=True, stop=False, skip_group_check=True
            )
            for j in range(4):
                for i in range(3):
                    off, w = J_OFF[j], J_W[j]
                    lhs = wt[:, i * PAIRW + off : i * PAIRW + off + 2 * w]
                    lhs = lhs.rearrange("p (c m) -> p c m", c=2)
                    dst = h[0:w, :]
                    xb = (j % 2) * NCH * L
                    rhs = xts[j // 2][
                        :, xb + 2 * i * L : xb + 2 * (i + 1) * L
                    ].rearrange("p (c l) -> p c l", c=2)
                    nc.tensor.matmul(
                        dst,
                        lhs,
                        rhs,
                        start=False,
                        stop=(j == 3 and i == 2),
                        skip_group_check=True,
                        perf_mode=mybir.MatmulPerfMode.DoubleRow,
                    )

            if cfg.get("debug"):
                hcp = spool.tile([P, L], F32, tag="hcp", name=f"hcp{q}")
                nc.vector.tensor_copy(hcp[:], h[:])
                oe.dma_start(hdbg[q], hcp[:])

            # --- GroupSwish: ((h+b1)*0.5) * (1 + tanh(sp*(h+b1)/2)) ---
            # device h is W1SC*(W1@x); scales in cb fold the 1/W1SC back in
            t = spool.tile([P, L], F32, tag="t", name=f"t{q}")
            nc.scalar.activation(
                t[:],
                h[:],
                mybir.ActivationFunctionType.Tanh,
                bias=cbt[:, CB_SPB1 + q : CB_SPB1 + q + 1],
                scale=cbt[:, CB_SPH + q : CB_SPH + q + 1],
            )
            u = spool.tile([P, L], F32, tag="u", name=f"u{q}")
            nc.vector.tensor_scalar(
                u[:],
                h[:],
                cbt[:, CB_B1 + q : CB_B1 + q + 1],
                0.5 / W1SC,
                op0=mybir.AluOpType.add,
                op1=mybir.AluOpType.mult,
            )
            sw = spool.tile([P, L], BF16, tag="sw", name=f"sw{q}")
            nc.vector.scalar_tensor_tensor(
                sw[:],
                t[:],
                1.0,
                u[:],
                op0=mybir.AluOpType.add,
                op1=mybir.AluOpType.mult,
            )

            # --- o = blockdiag(W2/1.1) @ swish : all 4 groups in one matmul ---
            o = ops.tile([4 * C, L], F32, tag="o", name=f"o{q}")
            nc.tensor.matmul(
                o[:],
                w2t[:, q * 4 * C : (q + 1) * 4 * C],
                sw[:],
                start=True,
                stop=True,
            )
            if cfg.get("debug"):
                ocp = spool.tile([4 * C, L], F32, tag="ocp", name=f"ocp{q}")
                nc.vector.tensor_copy(ocp[:], o[:])
                oe.dma_start(odbg[q], ocp[:])

            # --- softmax over [C, L] per group (no max subtraction) ---
            expo = spool.tile([4 * C, L], F32, tag="expo", name=f"e{q}")
            esum = spool.tile([4 * C, 1], F32, tag="esum", name=f"es{q}")
            nc.scalar.activation(
                expo[:],
                o[:],
                mybir.ActivationFunctionType.Exp,
                bias=cbt[: 4 * C, CB_B2 + q : CB_B2 + q + 1],
                scale=1.0,
                accum_out=esum[:],
            )
            tot = tps.tile([4, 1], F32, tag="tot", name=f"tot{q}")
            nc.tensor.matmul(
                tot[:],
                cbt[: 4 * C, CB_BLK : CB_BLK + 4],
                esum[:],
                start=True,
                stop=True,
            )
            inv = spool.tile([4, 1], F32, tag="inv", name=f"inv{q}")
            nc.vector.reciprocal(inv[:], tot[:])
            bc = tps.tile([4 * C, 1], F32, tag="bc", name=f"bc{q}")
            nc.tensor.matmul(
                bc[:],
                cbt[:4, CB_BCM : CB_BCM + 4 * C],
                inv[:],
                start=True,
                stop=True,
            )
            invc = spool.tile([4 * C, 1], F32, tag="invc", name=f"ic{q}")
            nc.vector.tensor_copy(invc[:], bc[:])
            res = spool.tile([4 * C, L], F32, tag="res", name=f"r{q}")
            nc.vector.tensor_scalar_mul(res[:], expo[:], invc[:, 0:1])

            oe.dma_start(
                out[4 * q : 4 * (q + 1)].rearrange("j c l -> (j c) l"),
                res[:],
            )

    nc.compile()
    return nc


def _marshal(x, W1, b1, beta, W2, b2, cfg=DEFAULT_CFG):
    """Full inputs -> list of per-core input dicts (all heavy reshapes here)."""
    xg = np.ascontiguousarray(x, dtype=np.float32).reshape(B, X, L)
    # xm[gp, p, jj*NCH*L + c*L + l] = x[2gp+jj, 128c+p, l]
    xmain = (
        xg[:, : NCH * P]
        .reshape(B // 2, 2, NCH, P, L)
        .transpose(0, 3, 1, 2, 4)
        .astype(FP8NP)
        .reshape(B // 2, P, 2 * NCH * L)
    )
    # xrem[gq, 16j+r, l] = x[4gq+j, 768+r, l]
    xrem = xg[:, NCH * P :].astype(FP8NP).reshape(B // 4, 4 * KREM, L)

    w1s = W1.astype(np.float32, copy=False) * np.float32(W1SC)
    w1T = w1s.transpose(0, 2, 1)  # [B, X, Z]
    w1ck = w1T[:, : NCH * P].reshape(B // 4, 4, NCH, P, Z)  # [gq, j, c, p, z]
    w1m = np.zeros((B // 4, P, WCOLS), np.float32)
    for i in range(3):
        for cc in range(2):
            c = 2 * i + cc
            for j in range(4):
                base = i * PAIRW + J_OFF[j] + cc * J_W[j] + 32 * j
                w1m[:, :, base : base + Z] = w1ck[:, j, c]
    w1m = w1m.astype(FP8NP)
    # w1r[gq, 16j+r, 32j+z] = W1SC*W1T[4gq+j, 768+r, z], else 0 (block diag)
    w1r = np.zeros((B // 4, 4 * KREM, P), FP8NP)
    w1T4 = w1T.reshape(B // 4, 4, X, Z)
    for j in range(4):
        w1r[:, KREM * j : KREM * (j + 1), Z * j : Z * (j + 1)] = w1T4[
            :, j, NCH * P :
        ].astype(FP8NP)

    # w2b[gq, 32j+z, 10j+c] = W2[4gq+j, c, z]/1.1, else 0 (block diagonal)
    w2s = (W2.astype(np.float32, copy=False) * np.float32(1.0 / 1.1)).transpose(
        0, 2, 1
    )  # [B, Z, C]
    w2blk = np.zeros((B // 4, P, 4 * C), BF16NP)
    w2s4 = w2s.reshape(B // 4, 4, Z, C)
    for j in range(4):
        w2blk[:, Z * j : Z * (j + 1), C * j : C * (j + 1)] = w2s4[:, j].astype(
            BF16NP
        )

    # const blob per core [128, CB_COLS] f32
    b1f = b1.astype(np.float32, copy=False)
    b2f = b2.astype(np.float32, copy=False)
    sp = np.log1p(np.exp(beta.astype(np.float64))).astype(np.float32)  # softplus
    blk = np.zeros((P, 4), np.float32)
    bcm = np.zeros((P, 4 * C), np.float32)
    for j in range(4):
        blk[C * j : C * (j + 1), j] = 1.0
        bcm[j, C * j : C * (j + 1)] = 1.0

    in_maps = []
    for core in range(NCORE):
        s = slice(core * GPC, (core + 1) * GPC)
        sq = slice(core * NQ, (core + 1) * NQ)
        cbc = np.zeros((P, CB_COLS), np.float32)
        # stacked [32j+z, q] views for this core's quads
        b1c = b1f[s].reshape(NQ, 4 * Z).T  # [128, NQ]
        spc = np.repeat(sp[s].reshape(NQ, 4), Z, axis=1).T * 0.5  # [128, NQ]
        cbc[:, CB_SPH : CB_SPH + NQ] = spc / np.float32(W1SC)
        cbc[:, CB_SPB1 : CB_SPB1 + NQ] = spc * b1c
        cbc[:, CB_B1 : CB_B1 + NQ] = b1c * np.float32(W1SC)
        cbc[: 4 * C, CB_B2 : CB_B2 + NQ] = b2f[s].reshape(NQ, 4 * C).T
        cbc[:, CB_BLK : CB_BLK + 4] = blk
        cbc[:, CB_BCM : CB_BCM + 4 * C] = bcm
        sp2 = slice(core * GPC // 2, (core + 1) * GPC // 2)
        in_maps.append(
            {
                "xm": xmain[sp2],
                "xr": xrem[sq],
                "w1m": w1m[sq],
                "w1r": w1r[sq],
                "w2b": np.ascontiguousarray(
                    w2blk[sq].transpose(1, 0, 2).reshape(P, NQ * 4 * C)
                ),
                "cb": cbc,
            }
        )
    return in_maps


def _run(in_maps, cfg=DEFAULT_CFG, trace=False, tmpdir=None):
    key = str(sorted(cfg.items()))
    if key not in _CACHE:
        _CACHE[key] = _build(cfg)
    return run_bass_kernel_spmd(
        _CACHE[key],
        in_maps,
        core_ids=list(range(NCORE)),
        trace=trace,
        tmpdir=tmpdir,
    )


_LAST = {}


def kernel(x, W1, b1, beta, W2, b2):
    in_maps = _marshal(x, W1, b1, beta, W2, b2)
    trace = bool(os.environ.get("KERNEL_TRACE"))
    r = _run(in_maps, trace=trace, tmpdir=os.environ.get("KERNEL_TRACE_DIR"))
    _LAST["results"] = r
    outs = [r.results[c]["out"].reshape(GPC, C * L) for c in range(NCORE)]
    return np.concatenate(outs, axis=0)
